# revision 30
# baseline (speedup 1.0000x reference)
"""Trainium2 Bass kernel for nn_Attention (B=1, N=4096, DIM=768, HEADS=12).

Strategy: 8-way sequence (query) parallelism with zero cross-core
communication. Every core redundantly computes K^T and V for the full
sequence (collectives measure ~50 GB/s effective intra-chip — slower
than recomputing), then runs flash-style attention for its own
512-query slice over all 12 heads, followed by the local output
projection. Host passes x pre-transposed so no on-device transposes
are needed anywhere.

Matmul dtypes: float32r (TF32-like full-rate fp32 path) for QKV
projections and scores; bf16 for exp(S), V_aug and the output
projection. Scores use 64x128 row-tiled PE pairs (two heads run
concurrently on the PE array halves). Softmax denominators come from a
ones-column appended to V (O^T psum rows [0:64]=out, row 64=denom).
"""

import os
import sys
from contextlib import ExitStack

import numpy as np

sys.path.insert(0, "/opt/trn_rl_repo")

import concourse.bass as bass  # noqa: E402
import concourse.tile as tile  # noqa: E402
from concourse import bacc, mybir  # noqa: E402
from concourse.bass_utils import run_bass_kernel_spmd  # noqa: E402

N_CORES = 8
DIM = 768
HEADS = 12
SEQ = 4096
DHEAD = 64
NQ = SEQ // N_CORES  # 512 queries per core
NPAIRS = HEADS // 2  # 6 head pairs
KT = DIM // 128  # 6 contraction tiles
F32 = mybir.dt.float32
F32R = mybir.dt.float32r
BF16 = mybir.dt.bfloat16

_CACHE = {}


def _build():
    nc = bacc.Bacc("TRN2", target_bir_lowering=False, debug=False, num_devices=N_CORES)

    xT = nc.dram_tensor("xT", [KT, 128, SEQ], F32R, kind="ExternalInput").ap()
    xqT = nc.dram_tensor("xqT", [KT, 128, NQ], F32R, kind="ExternalInput").ap()
    wq = nc.dram_tensor("wq", [KT, 128, DIM], F32R, kind="ExternalInput").ap()
    wk = nc.dram_tensor("wk", [KT, 128, DIM], F32R, kind="ExternalInput").ap()
    wv = nc.dram_tensor("wv", [KT, 128, DIM], F32R, kind="ExternalInput").ap()
    wo = nc.dram_tensor("wo", [NPAIRS, 128, DIM], BF16, kind="ExternalInput").ap()
    bo = nc.dram_tensor("bo", [DIM], F32, kind="ExternalInput").ap()
    out = nc.dram_tensor("out", [NQ, DIM], F32, kind="ExternalOutput").ap()

    # internal DRAM spills
    kT_dram = nc.dram_tensor("kT_dram", [HEADS, 64, SEQ], F32R).ap()
    # V_aug layout: [seq_tile, part(=key within tile), head, 65] bf16
    v_dram = nc.dram_tensor("v_dram", [SEQ // 128, 128, HEADS, DHEAD + 1], BF16).ap()
    rec_dram = nc.dram_tensor("rec_dram", [NPAIRS, 2, NQ], F32).ap()

    with ExitStack() as ctx:
        tc = ctx.enter_context(tile.TileContext(nc))

        persist = ctx.enter_context(tc.tile_pool(name="persist", bufs=1))
        vpool = ctx.enter_context(tc.tile_pool(name="vpool", bufs=2))
        v1_0 = vpool.tile([128, SEQ // 128, DHEAD + 1], BF16, tag="v1", name="v1_0")
        v2_0 = vpool.tile([128, SEQ // 128, DHEAD + 1], BF16, tag="v2", name="v2_0")
        qT_sb = [persist.tile([128, NQ], F32R, tag=f"qt{h}", name=f"qt{h}") for h in range(HEADS)]
        proj_sb = [persist.tile([128, NQ], BF16, tag=f"proj{j}", name=f"proj{j}") for j in range(NPAIRS)]
        # persistent K tiles (ping/pong per head-of-pair), upper halves zeroed
        ktiles = [persist.tile([128, SEQ], F32R, tag=f"ktile{i}", name=f"ktile{i}") for i in range(2)]
        ktiles2 = [persist.tile([128, SEQ], F32R, tag=f"ktile2_{i}", name=f"ktile2_{i}") for i in range(2)]
        with ExitStack() as zctx:
            # f32r zeros to blank upper contraction halves (f32r memset is not
            # a valid ISA op, so memset f32 then convert via DVE copy)
            zpool = zctx.enter_context(tc.tile_pool(name="zpool", bufs=1))
            zero_f = zpool.tile([64, SEQ], F32, tag="zero_f", name="zero_f")
            nc.vector.memset(zero_f[:], 0.0)
            zero_r = zpool.tile([64, SEQ], F32R, tag="zero_r", name="zero_r")
            nc.vector.tensor_copy(zero_r[:], zero_f[:])
            for h in range(HEADS):
                nc.vector.tensor_copy(qT_sb[h][DHEAD:128, :], zero_r[:, 0:NQ])
            for i in range(2):
                nc.vector.tensor_copy(ktiles[i][DHEAD:128, :], zero_r[:])
                nc.vector.tensor_copy(ktiles2[i][DHEAD:128, :], zero_r[:])

        # ---------------- Phase 1: projections ----------------
        with ExitStack() as p1:
            wpool = p1.enter_context(tc.tile_pool(name="wpool", bufs=1))
            evac = p1.enter_context(tc.tile_pool(name="evac", bufs=2))
            psum1 = p1.enter_context(tc.tile_pool(name="psum1", bufs=2, space="PSUM"))

            wk_sb = [wpool.tile([128, DIM], F32R, tag=f"wk{k}", name=f"wk{k}") for k in range(KT)]
            wv_sb = [wpool.tile([128, DIM], F32R, tag=f"wv{k}", name=f"wv{k}") for k in range(KT)]
            for k in range(KT):
                nc.sync.dma_start(out=wk_sb[k][:], in_=wk[k])
                nc.sync.dma_start(out=wv_sb[k][:], in_=wv[k])

            xpool = p1.enter_context(tc.tile_pool(name="xpool", bufs=2))

            xn_pre = {}

            def _load_slab(nch):
                xn = [xpool.tile([128, 512], F32R, tag=f"xn{k}", name=f"xn{k}") for k in range(KT)]
                for k in range(KT):
                    nc.sync.dma_start(out=xn[k][:], in_=xT[k][:, nch * 512:(nch + 1) * 512])
                xn_pre[nch] = xn

            _load_slab(0)

            # Q^T: computed as head pairs, then split per head into the
            # lower half of zero-padded per-head tiles. Its weights live in
            # a sub-scope released before the K/V streaming loop.
            with ExitStack() as qs:
                qpool = qs.enter_context(tc.tile_pool(name="qpool", bufs=1))
                wqp = qs.enter_context(tc.tile_pool(name="wqp", bufs=3))
                xq_sb = [qpool.tile([128, NQ], F32R, tag=f"xq{k}", name=f"xq{k}") for k in range(KT)]
                for k in range(KT):
                    nc.sync.dma_start(out=xq_sb[k][:], in_=xqT[k])
                for mt in range(NPAIRS):
                    wq_mt = wqp.tile([128, KT, 128], F32R, tag="wq_mt", name="wq_mt")
                    for k in range(KT):
                        nc.sync.dma_start(
                            out=wq_mt[:, k, :],
                            in_=wq[k][:, mt * 128:(mt + 1) * 128],
                        )
                    ps = psum1.tile([128, NQ], F32, tag="psq", name="psq")
                    for k in range(KT):
                        nc.tensor.matmul(
                            ps[:], wq_mt[:, k, :], xq_sb[k][:],
                            start=(k == 0), stop=(k == KT - 1),
                        )
                    qev = evac.tile([128, NQ], F32R, tag="qev", name="qev")
                    nc.vector.tensor_copy(qev[:], ps[:])
                    nc.gpsimd.dma_start(out=qT_sb[2 * mt][0:DHEAD, :], in_=qev[0:DHEAD, :])
                    nc.gpsimd.dma_start(out=qT_sb[2 * mt + 1][0:DHEAD, :], in_=qev[DHEAD:128, :])

            # K^T and V streamed over sequence chunks of 512
            for nch in range(SEQ // 512):
                if nch + 1 < SEQ // 512:
                    _load_slab(nch + 1)
                xn = xn_pre.pop(nch)
                for mt in range(NPAIRS):
                    ps = psum1.tile([128, 512], F32, tag="psk", name="psk")
                    for k in range(KT):
                        nc.tensor.matmul(
                            ps[:], wk_sb[k][:, mt * 128:(mt + 1) * 128], xn[k][:],
                            start=(k == 0), stop=(k == KT - 1),
                        )
                    kev = evac.tile([128, 512], F32R, tag="kev", name="kev")
                    nc.vector.tensor_copy(kev[:], ps[:])
                    nc.gpsimd.dma_start(
                        out=kT_dram[2 * mt][:, nch * 512:(nch + 1) * 512],
                        in_=kev[0:DHEAD, :],
                    )
                    nc.gpsimd.dma_start(
                        out=kT_dram[2 * mt + 1][:, nch * 512:(nch + 1) * 512],
                        in_=kev[DHEAD:128, :],
                    )
                    if mt == 0:
                        nc.gpsimd.dma_start(
                            out=ktiles[0][0:DHEAD, nch * 512:(nch + 1) * 512],
                            in_=kev[0:DHEAD, :],
                        )
                        nc.gpsimd.dma_start(
                            out=ktiles2[0][0:DHEAD, nch * 512:(nch + 1) * 512],
                            in_=kev[DHEAD:128, :],
                        )
                for st in range(4):
                    seq_tile = nch * 4 + st
                    ps = psum1.tile([128, DIM], F32, tag="psv", name="psv")
                    for k in range(KT):
                        lhs = xn[k][:, st * 128:(st + 1) * 128]
                        nc.tensor.matmul(ps[:, 0:512], lhs, wv_sb[k][:, 0:512],
                                         start=(k == 0), stop=(k == KT - 1))
                        nc.tensor.matmul(ps[:, 512:DIM], lhs, wv_sb[k][:, 512:DIM],
                                         start=(k == 0), stop=(k == KT - 1))
                    vev = evac.tile([128, HEADS, DHEAD + 1], BF16, tag="vev", name="vev")
                    nc.vector.tensor_copy(
                        vev[:, :, 0:DHEAD],
                        ps[:].rearrange("p (h d) -> p h d", h=HEADS),
                    )
                    nc.vector.memset(vev[:, :, DHEAD:DHEAD + 1], 1.0)
                    nc.gpsimd.dma_start(out=v_dram[seq_tile], in_=vev[:])
                    # pair-0 fast path: stage heads 0/1 straight into SBUF
                    nc.vector.tensor_copy(v1_0[:, seq_tile, :], vev[:, 0, :])
                    nc.vector.tensor_copy(v2_0[:, seq_tile, :], vev[:, 1, :])


        # prefetch output-projection weights early (tiny, avoids tail stall)
        wopool = ctx.enter_context(tc.tile_pool(name="wopool", bufs=1))
        wo_sb = [wopool.tile([128, DIM], BF16, tag=f"wo{k}", name=f"wo{k}") for k in range(NPAIRS)]
        for k in range(NPAIRS):
            nc.sync.dma_start(out=wo_sb[k][:], in_=wo[k])
        bias_sb = wopool.tile([128, DIM], F32, tag="bias", name="bias")
        bo_b = bass.AP(tensor=bo.tensor, offset=bo.offset, ap=[[0, 128]] + bo.ap)
        nc.sync.dma_start(out=bias_sb[:], in_=bo_b)

        # ---------------- Phase 2: attention ----------------
        with ExitStack() as p2:
            epool = p2.enter_context(tc.tile_pool(name="epool", bufs=4))
            npool = p2.enter_context(tc.tile_pool(name="npool", bufs=2))
            psS = p2.enter_context(tc.tile_pool(name="psS", bufs=3, space="PSUM"))
            psO = p2.enter_context(tc.tile_pool(name="psO", bufs=1, space="PSUM"))

            for j in range(NPAIRS):
                h1, h2 = 2 * j, 2 * j + 1
                kt1 = ktiles[j % 2]
                kt2 = ktiles2[j % 2]
                if j == 0:
                    v1, v2 = v1_0, v2_0
                else:
                    nc.sync.dma_start(out=kt1[0:DHEAD, :], in_=kT_dram[h1])
                    nc.sync.dma_start(out=kt2[0:DHEAD, :], in_=kT_dram[h2])
                    v1 = vpool.tile([128, SEQ // 128, DHEAD + 1], BF16, tag="v1", name="v1")
                    v2 = vpool.tile([128, SEQ // 128, DHEAD + 1], BF16, tag="v2", name="v2")
                    nc.sync.dma_start(out=v1[:], in_=v_dram[:, :, h1, :].rearrange("s p d -> p s d"))
                    nc.sync.dma_start(out=v2[:], in_=v_dram[:, :, h2, :].rearrange("s p d -> p s d"))

                accO1 = npool.tile([DHEAD + 1, NQ], F32, tag="accO1", name="accO1")
                accO2 = npool.tile([DHEAD + 1, NQ], F32, tag="accO2", name="accO2")
                pO1 = pO2 = None
                for g in range(16):  # groups of 2 key-tiles of 128 = 256 keys
                    if g % 8 == 0:
                        pO1 = psO.tile([DHEAD + 1, NQ], F32, tag="po1", name="pO1")
                        pO2 = psO.tile([DHEAD + 1, NQ], F32, tag="po2", name="pO2")
                    pS1 = psS.tile([128, 2, 512], F32, tag="ps", name="pS1")
                    pS2 = psS.tile([128, 2, 512], F32, tag="ps", name="pS2")
                    for i in range(2):
                        kb = g * 2 + i
                        nc.tensor.matmul(
                            pS1[:, i, :], kt1[:, kb * 128:(kb + 1) * 128],
                            qT_sb[h1][:], start=True, stop=True,
                        )
                        nc.tensor.matmul(
                            pS2[:, i, :], kt2[:, kb * 128:(kb + 1) * 128],
                            qT_sb[h2][:], start=True, stop=True,
                        )
                    e1 = epool.tile([128, 2, 512], BF16, tag="e1", name="e1")
                    e2 = epool.tile([128, 2, 512], BF16, tag="e2", name="e2")
                    nc.scalar.activation(e1[:], pS1[:], mybir.ActivationFunctionType.Exp)
                    nc.scalar.activation(e2[:], pS2[:], mybir.ActivationFunctionType.Exp)
                    for i in range(2):
                        kb = g * 2 + i
                        nc.tensor.matmul(pO1[:], v1[:, kb, :], e1[:, i, :],
                                         start=(kb % 16 == 0), stop=(kb % 16 == 15))
                        nc.tensor.matmul(pO2[:], v2[:, kb, :], e2[:, i, :],
                                         start=(kb % 16 == 0), stop=(kb % 16 == 15))
                    if g % 8 == 7:
                        # evacuate psum half into SBUF accumulators; frees the
                        # psum bank so the next half/pair can start immediately
                        if g == 7:
                            nc.vector.tensor_copy(accO1[:], pO1[:])
                            nc.vector.tensor_copy(accO2[:], pO2[:])
                        else:
                            nc.vector.tensor_add(accO1[:], accO1[:], pO1[:])
                            nc.vector.tensor_add(accO2[:], accO2[:], pO2[:])

                # normalize: recip of denominator rows; GpSimd broadcasts
                # partition 0 across partitions (GpSimd is otherwise idle)
                rec1 = npool.tile([1, NQ], F32, tag="rec1", name="rec1")
                rec2 = npool.tile([1, NQ], F32, tag="rec2", name="rec2")
                nc.vector.reciprocal(rec1[:], accO1[DHEAD:DHEAD + 1, :])
                nc.vector.reciprocal(rec2[:], accO2[DHEAD:DHEAD + 1, :])
                b1 = npool.tile([DHEAD, NQ], F32, tag="b1", name="b1")
                b2 = npool.tile([DHEAD, NQ], F32, tag="b2", name="b2")
                nc.gpsimd.partition_broadcast(b1[:], rec1[:])
                nc.gpsimd.partition_broadcast(b2[:], rec2[:])
                nc.vector.tensor_mul(proj_sb[j][0:DHEAD, :], accO1[0:DHEAD, :], b1[:])
                nc.vector.tensor_mul(proj_sb[j][DHEAD:128, :], accO2[0:DHEAD, :], b2[:])

        # ---------------- Phase 3: output projection ----------------
        with ExitStack() as p3:
            opool = p3.enter_context(tc.tile_pool(name="opool", bufs=2))
            psF = p3.enter_context(tc.tile_pool(name="psF", bufs=2, space="PSUM"))

            for qt in range(NQ // 128):
                ps = psF.tile([128, DIM], F32, tag="psf", name="psf")
                for k in range(NPAIRS):
                    lhs = proj_sb[k][:, qt * 128:(qt + 1) * 128]
                    nc.tensor.matmul(ps[:, 0:512], lhs, wo_sb[k][:, 0:512],
                                     start=(k == 0), stop=(k == NPAIRS - 1))
                    nc.tensor.matmul(ps[:, 512:DIM], lhs, wo_sb[k][:, 512:DIM],
                                     start=(k == 0), stop=(k == NPAIRS - 1))
                of = opool.tile([128, DIM], F32, tag="of", name="of")
                nc.vector.tensor_add(of[:], ps[:], bias_sb[:])
                nc.sync.dma_start(out=out[qt * 128:(qt + 1) * 128, :], in_=of[:])

    nc.compile()
    return nc


def kernel(x, W_qkv, W_out, b_out):
    import ml_dtypes

    if "nc" not in _CACHE:
        _CACHE["nc"] = _build()
    nc = _CACHE["nc"]

    x = np.asarray(x, dtype=np.float32)
    W_qkv = np.asarray(W_qkv, dtype=np.float32)
    W_out = np.asarray(W_out, dtype=np.float32)
    b_out = np.asarray(b_out, dtype=np.float32)

    xT = np.ascontiguousarray(x[0].T).reshape(KT, 128, SEQ)
    wq_h = np.ascontiguousarray(W_qkv[:, 0:DIM]).reshape(KT, 128, DIM)
    wk_h = np.ascontiguousarray(W_qkv[:, DIM:2 * DIM]).reshape(KT, 128, DIM)
    wv_h = np.ascontiguousarray(W_qkv[:, 2 * DIM:3 * DIM]).reshape(KT, 128, DIM)
    wo_h = np.ascontiguousarray(W_out.astype(ml_dtypes.bfloat16)).reshape(NPAIRS, 128, DIM)

    in_maps = []
    for c in range(N_CORES):
        xqT = np.ascontiguousarray(x[0, c * NQ:(c + 1) * NQ, :].T).reshape(KT, 128, NQ)
        in_maps.append({
            "xT": xT, "xqT": xqT, "wq": wq_h, "wk": wk_h, "wv": wv_h,
            "wo": wo_h, "bo": b_out,
        })

    res = run_bass_kernel_spmd(
        nc, in_maps, list(range(N_CORES)),
        trace=bool(os.environ.get("KERNEL_TRACE")),
    )
    _CACHE["last_exec_time_ns"] = res.exec_time_ns
    out = np.concatenate([res.results[c]["out"] for c in range(N_CORES)], axis=0)
    return out.reshape(1, SEQ, DIM)


# revision 31
# speedup vs baseline: 1.0108x; 1.0108x over previous
"""Trainium2 Bass kernel for nn_Attention (B=1, N=4096, DIM=768, HEADS=12).

Strategy: 8-way sequence (query) parallelism with zero cross-core
communication. Every core redundantly computes K^T and V for the full
sequence (collectives measure ~50 GB/s effective intra-chip — slower
than recomputing), then runs flash-style attention for its own
512-query slice over all 12 heads, followed by the local output
projection. Host passes x pre-transposed so no on-device transposes
are needed anywhere.

Matmul dtypes: float32r (TF32-like full-rate fp32 path) for QKV
projections and scores; bf16 for exp(S), V_aug and the output
projection. Scores use 64x128 row-tiled PE pairs (two heads run
concurrently on the PE array halves). Softmax denominators come from a
ones-column appended to V (O^T psum rows [0:64]=out, row 64=denom).
"""

import os
import sys
from contextlib import ExitStack

import numpy as np

sys.path.insert(0, "/opt/trn_rl_repo")

import concourse.bass as bass  # noqa: E402
import concourse.tile as tile  # noqa: E402
from concourse import bacc, mybir  # noqa: E402
from concourse.bass_utils import run_bass_kernel_spmd  # noqa: E402

N_CORES = 8
DIM = 768
HEADS = 12
SEQ = 4096
DHEAD = 64
NQ = SEQ // N_CORES  # 512 queries per core
NPAIRS = HEADS // 2  # 6 head pairs
KT = DIM // 128  # 6 contraction tiles
F32 = mybir.dt.float32
F32R = mybir.dt.float32r
BF16 = mybir.dt.bfloat16

_CACHE = {}


def _build():
    nc = bacc.Bacc("TRN2", target_bir_lowering=False, debug=False, num_devices=N_CORES)

    xT = nc.dram_tensor("xT", [KT, 128, SEQ], F32R, kind="ExternalInput").ap()
    xqT = nc.dram_tensor("xqT", [KT, 128, NQ], F32R, kind="ExternalInput").ap()
    wq = nc.dram_tensor("wq", [KT, 128, DIM], F32R, kind="ExternalInput").ap()
    wk = nc.dram_tensor("wk", [KT, 128, DIM], F32R, kind="ExternalInput").ap()
    wv = nc.dram_tensor("wv", [KT, 128, DIM], F32R, kind="ExternalInput").ap()
    wo = nc.dram_tensor("wo", [NPAIRS, 128, DIM], BF16, kind="ExternalInput").ap()
    bo = nc.dram_tensor("bo", [DIM], F32, kind="ExternalInput").ap()
    out = nc.dram_tensor("out", [NQ, DIM], F32, kind="ExternalOutput").ap()

    # internal DRAM spills
    kT_dram = nc.dram_tensor("kT_dram", [HEADS, 64, SEQ], F32R).ap()
    # V_aug layout: [seq_tile, part(=key within tile), head, 65] bf16
    v_dram = nc.dram_tensor("v_dram", [SEQ // 128, 128, HEADS, DHEAD + 1], BF16).ap()
    rec_dram = nc.dram_tensor("rec_dram", [NPAIRS, 2, NQ], F32).ap()

    with ExitStack() as ctx:
        tc = ctx.enter_context(tile.TileContext(nc))

        persist = ctx.enter_context(tc.tile_pool(name="persist", bufs=1))
        vpool = ctx.enter_context(tc.tile_pool(name="vpool", bufs=2))
        v1_0 = vpool.tile([128, SEQ // 128, DHEAD + 1], BF16, tag="v1", name="v1_0")
        v2_0 = vpool.tile([128, SEQ // 128, DHEAD + 1], BF16, tag="v2", name="v2_0")
        qT_sb = [persist.tile([128, NQ], F32R, tag=f"qt{h}", name=f"qt{h}") for h in range(HEADS)]
        proj_sb = [persist.tile([128, NQ], BF16, tag=f"proj{j}", name=f"proj{j}") for j in range(NPAIRS)]
        # persistent K tiles (ping/pong per head-of-pair), upper halves zeroed
        ktiles = [persist.tile([128, SEQ], F32R, tag=f"ktile{i}", name=f"ktile{i}") for i in range(2)]
        ktiles2 = [persist.tile([128, SEQ], F32R, tag=f"ktile2_{i}", name=f"ktile2_{i}") for i in range(2)]
        with ExitStack() as zctx:
            # f32r zeros to blank upper contraction halves (f32r memset is not
            # a valid ISA op, so memset f32 then convert via DVE copy)
            zpool = zctx.enter_context(tc.tile_pool(name="zpool", bufs=1))
            zero_f = zpool.tile([64, SEQ], F32, tag="zero_f", name="zero_f")
            nc.vector.memset(zero_f[:], 0.0)
            zero_r = zpool.tile([64, SEQ], F32R, tag="zero_r", name="zero_r")
            nc.vector.tensor_copy(zero_r[:], zero_f[:])
            for h in range(HEADS):
                nc.vector.tensor_copy(qT_sb[h][DHEAD:128, :], zero_r[:, 0:NQ])
            for i in range(2):
                nc.vector.tensor_copy(ktiles[i][DHEAD:128, :], zero_r[:])
                nc.vector.tensor_copy(ktiles2[i][DHEAD:128, :], zero_r[:])

        # ---------------- Phase 1: projections ----------------
        with ExitStack() as p1:
            wpool = p1.enter_context(tc.tile_pool(name="wpool", bufs=1))
            evac = p1.enter_context(tc.tile_pool(name="evac", bufs=2))
            psum1 = p1.enter_context(tc.tile_pool(name="psum1", bufs=2, space="PSUM"))

            wk_sb = [wpool.tile([128, DIM], F32R, tag=f"wk{k}", name=f"wk{k}") for k in range(KT)]
            wv_sb = [wpool.tile([128, DIM], F32R, tag=f"wv{k}", name=f"wv{k}") for k in range(KT)]
            for k in range(KT):
                nc.sync.dma_start(out=wk_sb[k][:], in_=wk[k])
                nc.sync.dma_start(out=wv_sb[k][:], in_=wv[k])

            xpool = p1.enter_context(tc.tile_pool(name="xpool", bufs=2))

            xn_pre = {}

            def _load_slab(nch):
                xn = [xpool.tile([128, 512], F32R, tag=f"xn{k}", name=f"xn{k}") for k in range(KT)]
                for k in range(KT):
                    nc.sync.dma_start(out=xn[k][:], in_=xT[k][:, nch * 512:(nch + 1) * 512])
                xn_pre[nch] = xn

            _load_slab(0)

            # Q^T: computed as head pairs, then split per head into the
            # lower half of zero-padded per-head tiles. Its weights live in
            # a sub-scope released before the K/V streaming loop.
            with ExitStack() as qs:
                qpool = qs.enter_context(tc.tile_pool(name="qpool", bufs=1))
                wqp = qs.enter_context(tc.tile_pool(name="wqp", bufs=3))
                xq_sb = [qpool.tile([128, NQ], F32R, tag=f"xq{k}", name=f"xq{k}") for k in range(KT)]
                for k in range(KT):
                    nc.sync.dma_start(out=xq_sb[k][:], in_=xqT[k])
                for mt in range(NPAIRS):
                    wq_mt = wqp.tile([128, KT, 128], F32R, tag="wq_mt", name="wq_mt")
                    for k in range(KT):
                        nc.sync.dma_start(
                            out=wq_mt[:, k, :],
                            in_=wq[k][:, mt * 128:(mt + 1) * 128],
                        )
                    ps = psum1.tile([128, NQ], F32, tag="psq", name="psq")
                    for k in range(KT):
                        nc.tensor.matmul(
                            ps[:], wq_mt[:, k, :], xq_sb[k][:],
                            start=(k == 0), stop=(k == KT - 1),
                        )
                    qev = evac.tile([128, NQ], F32R, tag="qev", name="qev")
                    nc.vector.tensor_copy(qev[:], ps[:])
                    nc.gpsimd.dma_start(out=qT_sb[2 * mt][0:DHEAD, :], in_=qev[0:DHEAD, :])
                    nc.gpsimd.dma_start(out=qT_sb[2 * mt + 1][0:DHEAD, :], in_=qev[DHEAD:128, :])

            # K^T and V streamed over sequence chunks of 512
            for nch in range(SEQ // 512):
                if nch + 1 < SEQ // 512:
                    _load_slab(nch + 1)
                xn = xn_pre.pop(nch)
                for mt in range(NPAIRS):
                    ps = psum1.tile([128, 512], F32, tag="psk", name="psk")
                    for k in range(KT):
                        nc.tensor.matmul(
                            ps[:], wk_sb[k][:, mt * 128:(mt + 1) * 128], xn[k][:],
                            start=(k == 0), stop=(k == KT - 1),
                        )
                    kev = evac.tile([128, 512], F32R, tag="kev", name="kev")
                    nc.vector.tensor_copy(kev[:], ps[:])
                    nc.gpsimd.dma_start(
                        out=kT_dram[2 * mt][:, nch * 512:(nch + 1) * 512],
                        in_=kev[0:DHEAD, :],
                    )
                    nc.gpsimd.dma_start(
                        out=kT_dram[2 * mt + 1][:, nch * 512:(nch + 1) * 512],
                        in_=kev[DHEAD:128, :],
                    )
                    if mt == 0:
                        nc.gpsimd.dma_start(
                            out=ktiles[0][0:DHEAD, nch * 512:(nch + 1) * 512],
                            in_=kev[0:DHEAD, :],
                        )
                        nc.gpsimd.dma_start(
                            out=ktiles2[0][0:DHEAD, nch * 512:(nch + 1) * 512],
                            in_=kev[DHEAD:128, :],
                        )
                for st in range(4):
                    seq_tile = nch * 4 + st
                    ps = psum1.tile([128, DIM], F32, tag="psv", name="psv")
                    for k in range(KT):
                        lhs = xn[k][:, st * 128:(st + 1) * 128]
                        nc.tensor.matmul(ps[:, 0:512], lhs, wv_sb[k][:, 0:512],
                                         start=(k == 0), stop=(k == KT - 1))
                        nc.tensor.matmul(ps[:, 512:DIM], lhs, wv_sb[k][:, 512:DIM],
                                         start=(k == 0), stop=(k == KT - 1))
                    vev = evac.tile([128, HEADS, DHEAD + 1], BF16, tag="vev", name="vev")
                    nc.vector.tensor_copy(
                        vev[:, :, 0:DHEAD],
                        ps[:].rearrange("p (h d) -> p h d", h=HEADS),
                    )
                    nc.vector.memset(vev[:, :, DHEAD:DHEAD + 1], 1.0)
                    nc.gpsimd.dma_start(out=v_dram[seq_tile], in_=vev[:])
                    # pair-0 fast path: stage heads 0/1 straight into SBUF
                    nc.vector.tensor_copy(v1_0[:, seq_tile, :], vev[:, 0, :])
                    nc.vector.tensor_copy(v2_0[:, seq_tile, :], vev[:, 1, :])


        # prefetch output-projection weights early (tiny, avoids tail stall)
        wopool = ctx.enter_context(tc.tile_pool(name="wopool", bufs=1))
        wo_sb = [wopool.tile([128, DIM], BF16, tag=f"wo{k}", name=f"wo{k}") for k in range(NPAIRS)]
        for k in range(NPAIRS):
            nc.sync.dma_start(out=wo_sb[k][:], in_=wo[k])
        bias_sb = wopool.tile([128, DIM], F32, tag="bias", name="bias")
        bo_b = bass.AP(tensor=bo.tensor, offset=bo.offset, ap=[[0, 128]] + bo.ap)
        nc.sync.dma_start(out=bias_sb[:], in_=bo_b)

        # ---------------- Phase 2: attention ----------------
        with ExitStack() as p2:
            epool = p2.enter_context(tc.tile_pool(name="epool", bufs=3))
            npool = p2.enter_context(tc.tile_pool(name="npool", bufs=2))
            psS = p2.enter_context(tc.tile_pool(name="psS", bufs=3, space="PSUM"))
            psO = p2.enter_context(tc.tile_pool(name="psO", bufs=1, space="PSUM"))

            for j in range(NPAIRS):
                h1, h2 = 2 * j, 2 * j + 1
                kt1 = ktiles[j % 2]
                kt2 = ktiles2[j % 2]
                if j == 0:
                    v1, v2 = v1_0, v2_0
                else:
                    nc.sync.dma_start(out=kt1[0:DHEAD, :], in_=kT_dram[h1])
                    nc.sync.dma_start(out=kt2[0:DHEAD, :], in_=kT_dram[h2])
                    v1 = vpool.tile([128, SEQ // 128, DHEAD + 1], BF16, tag="v1", name="v1")
                    v2 = vpool.tile([128, SEQ // 128, DHEAD + 1], BF16, tag="v2", name="v2")
                    nc.sync.dma_start(out=v1[:], in_=v_dram[:, :, h1, :].rearrange("s p d -> p s d"))
                    nc.sync.dma_start(out=v2[:], in_=v_dram[:, :, h2, :].rearrange("s p d -> p s d"))

                accO1 = npool.tile([DHEAD + 1, NQ], F32, tag="accO1", name="accO1")
                accO2 = npool.tile([DHEAD + 1, NQ], F32, tag="accO2", name="accO2")
                pO1 = pO2 = None
                for g in range(16):  # groups of 2 key-tiles of 128 = 256 keys
                    if g % 8 == 0:
                        pO1 = psO.tile([DHEAD + 1, NQ], F32, tag="po1", name="pO1")
                        pO2 = psO.tile([DHEAD + 1, NQ], F32, tag="po2", name="pO2")
                    pS1 = psS.tile([128, 2, 512], F32, tag="ps", name="pS1")
                    pS2 = psS.tile([128, 2, 512], F32, tag="ps", name="pS2")
                    for i in range(2):
                        kb = g * 2 + i
                        nc.tensor.matmul(
                            pS1[:, i, :], kt1[:, kb * 128:(kb + 1) * 128],
                            qT_sb[h1][:], start=True, stop=True,
                        )
                        nc.tensor.matmul(
                            pS2[:, i, :], kt2[:, kb * 128:(kb + 1) * 128],
                            qT_sb[h2][:], start=True, stop=True,
                        )
                    e1 = epool.tile([128, 2, 512], BF16, tag="e1", name="e1")
                    e2 = epool.tile([128, 2, 512], BF16, tag="e2", name="e2")
                    nc.scalar.activation(e1[:], pS1[:], mybir.ActivationFunctionType.Exp)
                    nc.scalar.activation(e2[:], pS2[:], mybir.ActivationFunctionType.Exp)
                    for i in range(2):
                        kb = g * 2 + i
                        nc.tensor.matmul(pO1[:], v1[:, kb, :], e1[:, i, :],
                                         start=(kb % 16 == 0), stop=(kb % 16 == 15))
                        nc.tensor.matmul(pO2[:], v2[:, kb, :], e2[:, i, :],
                                         start=(kb % 16 == 0), stop=(kb % 16 == 15))
                    if g % 8 == 7:
                        # evacuate psum half into SBUF accumulators; frees the
                        # psum bank so the next half/pair can start immediately
                        if g == 7:
                            nc.vector.tensor_copy(accO1[:], pO1[:])
                            nc.vector.tensor_copy(accO2[:], pO2[:])
                        else:
                            nc.vector.tensor_add(accO1[:], accO1[:], pO1[:])
                            nc.vector.tensor_add(accO2[:], accO2[:], pO2[:])

                # normalize: recip of denominator rows; GpSimd broadcasts
                # partition 0 across partitions (GpSimd is otherwise idle)
                rec1 = npool.tile([1, NQ], F32, tag="rec1", name="rec1")
                rec2 = npool.tile([1, NQ], F32, tag="rec2", name="rec2")
                nc.vector.reciprocal(rec1[:], accO1[DHEAD:DHEAD + 1, :])
                nc.vector.reciprocal(rec2[:], accO2[DHEAD:DHEAD + 1, :])
                b1 = npool.tile([DHEAD, NQ], F32, tag="b1", name="b1")
                b2 = npool.tile([DHEAD, NQ], F32, tag="b2", name="b2")
                nc.gpsimd.partition_broadcast(b1[:], rec1[:])
                nc.gpsimd.partition_broadcast(b2[:], rec2[:])
                nc.vector.tensor_mul(proj_sb[j][0:DHEAD, :], accO1[0:DHEAD, :], b1[:])
                nc.vector.tensor_mul(proj_sb[j][DHEAD:128, :], accO2[0:DHEAD, :], b2[:])

        # ---------------- Phase 3: output projection ----------------
        with ExitStack() as p3:
            opool = p3.enter_context(tc.tile_pool(name="opool", bufs=2))
            psF = p3.enter_context(tc.tile_pool(name="psF", bufs=2, space="PSUM"))

            for qt in range(NQ // 128):
                ps = psF.tile([128, DIM], F32, tag="psf", name="psf")
                for k in range(NPAIRS):
                    lhs = proj_sb[k][:, qt * 128:(qt + 1) * 128]
                    nc.tensor.matmul(ps[:, 0:512], lhs, wo_sb[k][:, 0:512],
                                     start=(k == 0), stop=(k == NPAIRS - 1))
                    nc.tensor.matmul(ps[:, 512:DIM], lhs, wo_sb[k][:, 512:DIM],
                                     start=(k == 0), stop=(k == NPAIRS - 1))
                of = opool.tile([128, DIM], F32, tag="of", name="of")
                nc.vector.tensor_add(of[:], ps[:], bias_sb[:])
                nc.sync.dma_start(out=out[qt * 128:(qt + 1) * 128, :], in_=of[:])

    nc.compile()
    return nc


def kernel(x, W_qkv, W_out, b_out):
    import ml_dtypes

    if "nc" not in _CACHE:
        _CACHE["nc"] = _build()
    nc = _CACHE["nc"]

    x = np.asarray(x, dtype=np.float32)
    W_qkv = np.asarray(W_qkv, dtype=np.float32)
    W_out = np.asarray(W_out, dtype=np.float32)
    b_out = np.asarray(b_out, dtype=np.float32)

    xT = np.ascontiguousarray(x[0].T).reshape(KT, 128, SEQ)
    wq_h = np.ascontiguousarray(W_qkv[:, 0:DIM]).reshape(KT, 128, DIM)
    wk_h = np.ascontiguousarray(W_qkv[:, DIM:2 * DIM]).reshape(KT, 128, DIM)
    wv_h = np.ascontiguousarray(W_qkv[:, 2 * DIM:3 * DIM]).reshape(KT, 128, DIM)
    wo_h = np.ascontiguousarray(W_out.astype(ml_dtypes.bfloat16)).reshape(NPAIRS, 128, DIM)

    in_maps = []
    for c in range(N_CORES):
        xqT = np.ascontiguousarray(x[0, c * NQ:(c + 1) * NQ, :].T).reshape(KT, 128, NQ)
        in_maps.append({
            "xT": xT, "xqT": xqT, "wq": wq_h, "wk": wk_h, "wv": wv_h,
            "wo": wo_h, "bo": b_out,
        })

    res = run_bass_kernel_spmd(
        nc, in_maps, list(range(N_CORES)),
        trace=bool(os.environ.get("KERNEL_TRACE")),
    )
    _CACHE["last_exec_time_ns"] = res.exec_time_ns
    out = np.concatenate([res.results[c]["out"] for c in range(N_CORES)], axis=0)
    return out.reshape(1, SEQ, DIM)
